# revision 43
# baseline (speedup 1.0000x reference)
"""Trainium2 Bass kernel for nn_MultiHeadAttention (B=2, S=2048, D=1024, H=16).

Sharding: 8 cores = 2 batches x 4 head-groups (4 heads each).
Each core receives host-preshuffled activations x^T ([P, KC, S] layout,
fp8e4 to halve the input stream) plus its head-group's weight slices (bf16).

Per core:
  stage A (interleaved by q-chunk with the input DMA stream):
    Q^T,K^T = W^T x^T  (per-head [DK, S] bf16, head pair stacked on parts)
    V_aug   = [x Wv + bv, 1/g]  (natural [S, DK+1] per head; column DK holds
      the reciprocal of the per-head sigmoid gate, device-computed and
      written once via gpsimd partition_broadcast)
    pooled means on DVE; gate = sigmoid(pooled @ Wg)
  stage B: flat stream of (q-chunk, head-pair) blocks with a global
    2-deep AV software pipeline that crosses block boundaries:
      scores: two contract-64 matmuls as concurrent PE row-tiles into two
        banks of one PSUM tile -> ONE joint Exp [128, 2, Ni] on scalar;
        causal diagonal masked on DVE.
      normalization chain for block k emitted at block k+1 step 1
        (copies->hop->one [2,512] reciprocal->gpsimd ring broadcast),
        multiplies at block k+2 step 1, stage C s-tile k at block k+3 end.
    gpsimd runs ONLY partition_broadcast (no ucode library thrash).
  stage C: interleaved output projection (host sums 4 partials + bo).
"""

import numpy as np

P = 128
CHUNK = 512

_BUILD_CACHE = {}


def _build(S, D, DOUT, HPC, DK, causal):
    import concourse.bass as bass
    import concourse.mybir as mybir
    import concourse.tile as tile
    from concourse import bacc
    from concourse.bass import ds, ts

    fp32 = mybir.dt.float32
    bf16 = mybir.dt.bfloat16
    fp8 = mybir.dt.float8e4
    KC = D // P             # contraction k-chunks for projections
    GCOLS = HPC * DK        # this core's projection output width
    MT = GCOLS // P         # head-pair tiles (2 heads of DK=64 per tile)
    NCH = S // CHUNK        # q-chunks
    TPC = CHUNK // P        # kv tiles per q-chunk (4)
    NKV = S // P            # kv tiles total
    KC2 = GCOLS // P        # out-proj contraction chunks
    NOC = DOUT // CHUNK     # out-proj N chunks
    ST = S // P             # s-tiles
    HALF = S // 2
    assert DK * 2 == P and GCOLS % P == 0 and NCH % 2 == 0
    assert NOC == 2, "stage C packs its two N-chunks into one 2-bank tile"

    Act = mybir.ActivationFunctionType
    nc = bacc.Bacc()

    xq_d = nc.declare_dram_parameter("xq", [P, KC, S], bf16, isOutput=False)
    xk_d = nc.declare_dram_parameter("xk", [P, KC, S], bf16, isOutput=False)
    xv_d = nc.declare_dram_parameter("xv", [P, KC, S], bf16, isOutput=False)
    wq_d = nc.declare_dram_parameter("wq", [P, KC, GCOLS], bf16, isOutput=False)
    wk_d = nc.declare_dram_parameter("wk", [P, KC, GCOLS], bf16, isOutput=False)
    wv_d = nc.declare_dram_parameter("wv", [P, KC, GCOLS], bf16, isOutput=False)
    wo_d = nc.declare_dram_parameter("wo", [P, KC2, DOUT], bf16, isOutput=False)
    bq_d = nc.declare_dram_parameter("bq", [P, MT], fp32, isOutput=False)
    bk_d = nc.declare_dram_parameter("bk", [P, MT], fp32, isOutput=False)
    bv_d = nc.declare_dram_parameter("bv", [1, GCOLS], bf16, isOutput=False)
    wgq_d = nc.declare_dram_parameter("wgq", [P, KC, HPC], fp32, isOutput=False)
    wgk_d = nc.declare_dram_parameter("wgk", [P, KC, HPC], fp32, isOutput=False)
    bg_d = nc.declare_dram_parameter("bg", [1, HPC], fp32, isOutput=False)
    mtri_d = nc.declare_dram_parameter("mtri", [P, P], bf16, isOutput=False)
    outp = nc.declare_dram_parameter("out", [S, DOUT], fp32, isOutput=True)

    scale = 1.0 / float(np.sqrt(DK))
    DK1 = DK + 1

    with tile.TileContext(nc) as tc:
        with (
            tc.tile_pool(name="persist", bufs=1) as pp,
            tc.tile_pool(name="wts", bufs=1) as wp,
        ):
            xq = pp.tile([P, KC, S], bf16, tag="xq")
            xk = pp.tile([P, KC, S], bf16, tag="xk")
            xv = pp.tile([P, KC, S], bf16, tag="xv")
            qt = pp.tile([P, MT, S], bf16, tag="qt")
            kt = pp.tile([P, MT, S], bf16, tag="kt")
            vaug = pp.tile([P, ST, HPC, DK1], bf16, tag="vaug")
            hcat = pp.tile([P, KC2, S], bf16, tag="hcat")
            ones = pp.tile([P, P], fp32, tag="ones")
            nc.any.memset(ones[:], 1.0)
            ones_bf = pp.tile([1, P], bf16, tag="ones_bf")
            nc.any.memset(ones_bf[:], 1.0)

            wq = wp.tile([P, KC, GCOLS], bf16, tag="wq")
            wk = wp.tile([P, KC, GCOLS], bf16, tag="wk")
            wv = wp.tile([P, KC, GCOLS], bf16, tag="wv")
            wo = wp.tile([P, KC2, DOUT], bf16, tag="wo")
            bq = wp.tile([P, MT], fp32, tag="bq")
            bk = wp.tile([P, MT], fp32, tag="bk")
            bv = wp.tile([1, GCOLS], bf16, tag="bv")
            wgq = wp.tile([P, KC, HPC], fp32, tag="wgq")
            wgk = wp.tile([P, KC, HPC], fp32, tag="wgk")
            bg = wp.tile([1, HPC], fp32, tag="bg")
            mtri = wp.tile([P, P], bf16, tag="mtri")

            # ---- DMA issue order: what the first matmuls need comes first,
            # each big tensor split across several queues.
            for c in range(0, KC, 2):
                nc.sync.dma_start(wq[:, c : c + 2, :], wq_d[:, c : c + 2, :])
            nc.sync.dma_start(bq[:], bq_d[:])
            nc.sync.dma_start(mtri[:], mtri_d[:])
            for c in range(0, KC, 2):
                nc.sync.dma_start(wk[:, c : c + 2, :], wk_d[:, c : c + 2, :])
            nc.sync.dma_start(bk[:], bk_d[:])
            for c in range(KC):
                nc.sync.dma_start(xq[:, c, 0:HALF], xq_d[:, c, 0:HALF])
            for c in range(KC):
                nc.sync.dma_start(xk[:, c, 0:HALF], xk_d[:, c, 0:HALF])
            for c in range(0, KC, 2):
                nc.sync.dma_start(wv[:, c : c + 2, :], wv_d[:, c : c + 2, :])
            nc.sync.dma_start(bv[:], bv_d[:])
            for c in range(KC):
                nc.sync.dma_start(xv[:, c, 0:HALF], xv_d[:, c, 0:HALF])
            for c in range(KC):
                nc.sync.dma_start(xq[:, c, HALF:S], xq_d[:, c, HALF:S])
            for c in range(KC):
                nc.sync.dma_start(xk[:, c, HALF:S], xk_d[:, c, HALF:S])
            for c in range(KC):
                nc.sync.dma_start(xv[:, c, HALF:S], xv_d[:, c, HALF:S])
            for c in range(KC2):
                nc.sync.dma_start(wo[:, c, :], wo_d[:, c, :])
            nc.sync.dma_start(wgq[:], wgq_d[:])
            nc.sync.dma_start(wgk[:], wgk_d[:])
            nc.sync.dma_start(bg[:], bg_d[:])

            # ---------------- Stage A: projections + pooled means + gate,
            # interleaved across Q/K/V by q-chunk to match DMA arrival.
            with (
                tc.tile_pool(name="psa", bufs=3, space="PSUM") as psa,
                tc.tile_pool(name="pmisc", bufs=1) as pm,
            ):
                pooled_q = pm.tile([P, KC], fp32, tag="pq")
                pooled_k = pm.tile([P, KC], fp32, tag="pk")
                scratch = pm.tile([P, S], bf16, tag="scratch")

                def proj_chunk(x_sb, w_sb, b_sb, out_sb, n):
                    nsl = ds(n * CHUNK, CHUNK)
                    for m in range(MT):
                        ps = psa.tile([P, CHUNK], fp32, tag="psa", bufs=3)
                        for k in range(KC):
                            nc.tensor.matmul(
                                ps[:], w_sb[:, k, ts(m, P)], x_sb[:, k, nsl],
                                start=(k == 0), stop=(k == KC - 1))
                        nc.vector.tensor_scalar_add(
                            out_sb[:, m, nsl], ps[:], b_sb[:, m : m + 1])

                def v_tile(st):
                    ps = psa.tile([P, GCOLS], fp32, tag="psv", bufs=2)
                    for k in range(KC):
                        nc.tensor.matmul(
                            ps[:], xv[:, k, ts(st, P)], wv[:, k, :],
                            start=(k == 0), stop=False)
                    nc.tensor.matmul(
                        ps[:], ones_bf[0:1, 0:P], bv[:], start=False, stop=True)
                    nc.vector.tensor_copy(
                        vaug[:, st, :, 0:DK],
                        ps.rearrange("p (h d) -> p h d", d=DK))

                for n in range(NCH):
                    proj_chunk(xq, wq, bq, qt, n)
                    proj_chunk(xk, wk, bk, kt, n)
                    if n == NCH - 1:
                        for k in range(KC):
                            nc.scalar.activation(
                                scratch[:], xq[:, k, :], Act.Identity,
                                accum_out=pooled_q[:, k : k + 1])
                        for k in range(KC):
                            nc.scalar.activation(
                                scratch[:], xk[:, k, :], Act.Identity,
                                accum_out=pooled_k[:, k : k + 1])
                    for st in range(n * TPC, (n + 1) * TPC):
                        v_tile(st)

                # gate logits -> sigmoid -> 1/g -> vaug column DK
                psg = psa.tile([1, HPC], fp32, tag="psg", bufs=1)
                for k in range(KC):
                    nc.tensor.matmul(psg[:], pooled_q[:, k : k + 1], wgq[:, k, :],
                                     start=(k == 0), stop=False)
                for k in range(KC):
                    nc.tensor.matmul(psg[:], pooled_k[:, k : k + 1], wgk[:, k, :],
                                     start=False, stop=False)
                nc.tensor.matmul(psg[:], ones[0:1, 0:1], bg[:],
                                 start=False, stop=True)
                gate0 = pm.tile([1, HPC], fp32, tag="gate0")
                nc.scalar.activation(gate0[:], psg[:], Act.Sigmoid)
                invg = pm.tile([1, ST, HPC], bf16, tag="invg")
                invgf = pm.tile([1, 1, HPC], fp32, tag="invgf")
                nc.vector.reciprocal(invgf[0:1, 0, :], gate0[0:1, :])
                nc.vector.tensor_copy(
                    invg[0:1, :, :], invgf[:].to_broadcast([1, ST, HPC]))
                nc.gpsimd.partition_broadcast(
                    vaug[:, :, :, DK:DK1], invg[0:1, :, :])

            # ---------------- Stage B + interleaved stage C
            with (
                tc.tile_pool(name="attn", bufs=3) as ap_,
                tc.tile_pool(name="rows", bufs=2) as rp,
                tc.tile_pool(name="otmp", bufs=2) as op_,
                tc.tile_pool(name="pssc", bufs=2, space="PSUM") as pssc,
                tc.tile_pool(name="psav", bufs=2, space="PSUM") as psav,
            ):
                blocks = []
                for j in range(NCH):
                    nkv_j = min(TPC * (j + 1), NKV) if causal else NKV
                    for hp in range(MT):
                        blocks.append((j, hp, nkv_j, ds(j * CHUNK, CHUNK)))

                def emit_av(item):
                    i, at, qrel, Ni, pe, po, hp, nkv_j = item
                    nc.tensor.matmul(
                        pe[:, ds(qrel, Ni)], vaug[:, i, 2 * hp, :],
                        at[:, 0, :Ni], start=(i == 0), stop=(i == nkv_j - 1))
                    nc.tensor.matmul(
                        po[:, ds(qrel, Ni)], vaug[:, i, 2 * hp + 1, :],
                        at[:, 1, :Ni], start=(i == 0), stop=(i == nkv_j - 1))

                def emit_chain(st8):
                    # den rows -> partitions 0/1 -> one [2,512] reciprocal ->
                    # ring broadcast; everything data-ready when reached.
                    (pe, po), _, _, _ = st8["av"]
                    rstk = rp.tile([P, 6, CHUNK], fp32, tag="rr", bufs=2)
                    for half, pav in ((0, pe), (1, po)):
                        nc.vector.tensor_copy(rstk[DK : DK1, half, :],
                                              pav[DK : DK1, :])
                        nc.sync.dma_start(rstk[half : half + 1, 2, :],
                                          rstk[DK : DK1, half, :])
                    nc.vector.reciprocal(rstk[0:2, 3, :], rstk[0:2, 2, :])
                    nc.sync.dma_start(rstk[0:1, 4, :], rstk[1:2, 3, :])
                    bcs = []
                    for half in (0, 1):
                        bc = rp.tile([DK, CHUNK], fp32, tag="bcs", bufs=2)
                        nc.gpsimd.partition_broadcast(
                            bc[:], rstk[0:1, 3 + half, :])
                        bcs.append(bc)
                    st8["bcs"] = bcs

                def emit_muls(st8):
                    (pe, po), hp, jsl, _ = st8["av"]
                    bcs = st8["bcs"]
                    nc.vector.tensor_mul(hcat[0:DK, hp, jsl],
                                         pe[0:DK, :], bcs[0]
                                         [:])
                    ot = op_.tile([DK, CHUNK], bf16, tag="ot", bufs=2)
                    nc.vector.tensor_mul(ot[:], po[0:DK, :], bcs[1][:])
                    nc.sync.dma_start(hcat[DK:P, hp, jsl], ot[:])

                hist = []      # per-block state dicts, newest last
                for bidx, (j, hp, nkv_j, jsl) in enumerate(blocks):
                    pe = psav.tile([DK1, CHUNK], fp32, tag="av_e", bufs=2)
                    po = psav.tile([DK1, CHUNK], fp32, tag="av_o", bufs=2)
                    st8 = {"av": ((pe, po), hp, jsl, nkv_j)}
                    avq = []
                    for i in range(nkv_j):
                        t = i - TPC * j
                        diag = causal and t >= 0
                        if diag:
                            Ni = CHUNK - P * t
                            qoff = j * CHUNK + P * t
                        else:
                            Ni = CHUNK
                            qoff = j * CHUNK
                        sc = pssc.tile([P, 2, CHUNK], fp32, tag="sc", bufs=2)
                        nc.tensor.matmul(
                            sc[:, 0, :Ni], kt[0:DK, hp, ts(i, P)],
                            qt[0:DK, hp, ds(qoff, Ni)], start=True, stop=True)
                        nc.tensor.matmul(
                            sc[:, 1, :Ni], kt[DK:P, hp, ts(i, P)],
                            qt[DK:P, hp, ds(qoff, Ni)], start=True, stop=True)
                        at = ap_.tile([P, 2, CHUNK], bf16, tag="at", bufs=3)
                        nc.scalar.activation(at[:, :, :Ni], sc[:, :, :Ni],
                                             Act.Exp, scale=scale)
                        if diag:
                            nc.vector.tensor_mul(
                                at[:, 0, 0:P], at[:, 0, 0:P], mtri[:])
                            nc.vector.tensor_mul(
                                at[:, 1, 0:P], at[:, 1, 0:P], mtri[:])
                        if len(avq) == 1:
                            emit_av(avq.pop(0))
                        avq.append((i, at, qoff - j * CHUNK, Ni,
                                    pe, po, hp, nkv_j))
                    while avq:
                        emit_av(avq.pop(0))
                    emit_chain(st8)
                    if hist:
                        emit_muls(hist[-1])
                    hist.append(st8)
                def stage_c(st):
                    # shares the "sc" psum tag (safe in the drain); both
                    # N-chunks in one 2-bank allocation
                    ps = pssc.tile([P, 2, CHUNK], fp32, tag="sc", bufs=2)
                    osb = op_.tile([P, DOUT], fp32, tag="osb", bufs=3)
                    for nh in range(NOC):
                        for k2 in range(KC2):
                            nc.tensor.matmul(
                                ps[:, nh, :], hcat[:, k2, ts(st, P)],
                                wo[:, k2, ds(nh * CHUNK, CHUNK)],
                                start=(k2 == 0), stop=(k2 == KC2 - 1))
                    nc.scalar.copy(osb[:, 0:CHUNK], ps[:, 0, :])
                    nc.vector.tensor_copy(osb[:, CHUNK:DOUT], ps[:, 1, :])
                    nc.sync.dma_start(outp[ts(st, P), :], osb[:])

                for st in range(ST - TPC):
                    stage_c(st)
                emit_muls(hist[-1])
                for st in range(ST - TPC, ST):
                    stage_c(st)

    nc.compile()
    return nc


def _prep_core_inputs(query, key_, value, Wq, bq, Wk, bk, Wv, bv, Wg, bg, Wo,
                      b, g, S, D, HPC, DK):
    import ml_dtypes
    GCOLS = HPC * DK
    KC = D // P
    KC2 = GCOLS // P
    MT = GCOLS // P
    H0 = g * HPC
    cs = slice(H0 * DK, H0 * DK + GCOLS)
    f32 = np.float32
    bf16 = ml_dtypes.bfloat16
    fp8 = ml_dtypes.float8_e4m3
    c = np.ascontiguousarray

    def shuf_x(x, dt):
        # [S, D] -> [P, KC, S] with [p, k, s] = x[s, k*P+p]
        return c(x.T.reshape(KC, P, S).transpose(1, 0, 2).astype(dt))

    def shuf_w(W):
        # [D, GCOLS] -> [P, KC, GCOLS]
        return c(W.reshape(KC, P, -1).transpose(1, 0, 2).astype(bf16))

    return {
        "xq": shuf_x(query[b], bf16),
        "xk": shuf_x(key_[b], bf16),
        "xv": shuf_x(value[b], bf16),
        "wq": shuf_w(Wq[:, cs]),
        "wk": shuf_w(Wk[:, cs]),
        "wv": shuf_w(Wv[:, cs]),
        "wo": c(Wo[cs, :].reshape(KC2, P, -1).transpose(1, 0, 2).astype(bf16)),
        "bq": c(bq[cs].reshape(MT, P).T.astype(f32)),
        "bk": c(bk[cs].reshape(MT, P).T.astype(f32)),
        "bv": c(bv[cs].astype(bf16)[None, :]),
        "wgq": c((Wg[:D, H0 : H0 + HPC] / S).reshape(KC, P, HPC)
                 .transpose(1, 0, 2).astype(f32)),
        "wgk": c((Wg[D:, H0 : H0 + HPC] / S).reshape(KC, P, HPC)
                 .transpose(1, 0, 2).astype(f32)),
        "bg": c(bg[H0 : H0 + HPC].astype(f32)[None, :]),
        "mtri": np.triu(np.ones((P, P), bf16)),
    }


_last_results = None


def kernel(query, key_, value, mask, Wq, bq, Wk, bk, Wv, bv, Wo, bo, Wg, bg):
    global _last_results
    from concourse.bass_utils import run_bass_kernel_spmd

    query = np.asarray(query)
    key_ = np.asarray(key_)
    value = np.asarray(value)
    mask = np.asarray(mask)
    B, S, D = query.shape
    H = np.asarray(bg).shape[0]
    DK = D // H
    DOUT = np.asarray(Wo).shape[1]
    NC_ = 8
    GROUPS = NC_ // B
    HPC = H // GROUPS

    causal = bool(
        np.array_equal(mask[0, 0], np.tril(np.ones((S, S), bool)))
    )
    if not causal:
        assert mask.all(), "only causal or all-true masks supported"

    key = (S, D, DOUT, HPC, DK, causal)
    if key not in _BUILD_CACHE:
        _BUILD_CACHE[key] = _build(*key)
    nc = _BUILD_CACHE[key]

    in_maps = []
    for cidx in range(NC_):
        b, gidx = divmod(cidx, GROUPS)
        in_maps.append(_prep_core_inputs(
            query, key_, value, Wq, bq, Wk, bk, Wv, bv, Wg, bg, Wo,
            b, gidx, S, D, HPC, DK))

    res = run_bass_kernel_spmd(nc, in_maps, core_ids=list(range(NC_)))
    _last_results = res

    out = np.zeros((B, S, DOUT), np.float32)
    for cidx in range(NC_):
        b = cidx // GROUPS
        out[b] += res.results[cidx]["out"]
    out += np.asarray(bo).astype(np.float32)
    return out


# revision 44
# speedup vs baseline: 1.1358x; 1.1358x over previous
"""Trainium2 Bass kernel for nn_MultiHeadAttention (B=2, S=2048, D=1024, H=16).

Sharding: 8 cores = 2 batches x 4 head-groups (4 heads each).
Each core receives host-preshuffled activations x^T ([P, KC, S] layout,
fp8e4 to halve the input stream) plus its head-group's weight slices (bf16).

Per core:
  stage A (interleaved by q-chunk with the input DMA stream):
    Q^T,K^T = W^T x^T  (per-head [DK, S] bf16, head pair stacked on parts)
    V_aug   = [x Wv + bv, 1/g]  (natural [S, DK+1] per head; column DK holds
      the reciprocal of the per-head sigmoid gate, device-computed and
      written once via gpsimd partition_broadcast)
    pooled means on DVE; gate = sigmoid(pooled @ Wg)
  stage B: flat stream of (q-chunk, head-pair) blocks with a global
    2-deep AV software pipeline that crosses block boundaries:
      scores: two contract-64 matmuls as concurrent PE row-tiles into two
        banks of one PSUM tile -> ONE joint Exp [128, 2, Ni] on scalar;
        causal diagonal masked on DVE.
      normalization chain for block k emitted at block k+1 step 1
        (copies->hop->one [2,512] reciprocal->gpsimd ring broadcast),
        multiplies at block k+2 step 1, stage C s-tile k at block k+3 end.
    gpsimd runs ONLY partition_broadcast (no ucode library thrash).
  stage C: interleaved output projection (host sums 4 partials + bo).
"""

import numpy as np

P = 128
CHUNK = 512

_BUILD_CACHE = {}


def _build(S, D, DOUT, HPC, DK, causal):
    import concourse.bass as bass
    import concourse.mybir as mybir
    import concourse.tile as tile
    from concourse import bacc
    from concourse.bass import ds, ts

    fp32 = mybir.dt.float32
    bf16 = mybir.dt.bfloat16
    fp8 = mybir.dt.float8e4
    KC = D // P             # contraction k-chunks for projections
    GCOLS = HPC * DK        # this core's projection output width
    MT = GCOLS // P         # head-pair tiles (2 heads of DK=64 per tile)
    NCH = S // CHUNK        # q-chunks
    TPC = CHUNK // P        # kv tiles per q-chunk (4)
    NKV = S // P            # kv tiles total
    KC2 = GCOLS // P        # out-proj contraction chunks
    NOC = DOUT // CHUNK     # out-proj N chunks
    ST = S // P             # s-tiles
    HALF = S // 2
    assert DK * 2 == P and GCOLS % P == 0 and NCH % 2 == 0
    assert NOC == 2, "stage C packs its two N-chunks into one 2-bank tile"

    Act = mybir.ActivationFunctionType
    nc = bacc.Bacc()

    xq_d = nc.declare_dram_parameter("xq", [P, KC, S], bf16, isOutput=False)
    xk_d = nc.declare_dram_parameter("xk", [P, KC, S], bf16, isOutput=False)
    xv_d = nc.declare_dram_parameter("xv", [P, KC, S], bf16, isOutput=False)
    wq_d = nc.declare_dram_parameter("wq", [P, KC, GCOLS], bf16, isOutput=False)
    wk_d = nc.declare_dram_parameter("wk", [P, KC, GCOLS], bf16, isOutput=False)
    wv_d = nc.declare_dram_parameter("wv", [P, KC, GCOLS], bf16, isOutput=False)
    wo_d = nc.declare_dram_parameter("wo", [P, KC2, DOUT], bf16, isOutput=False)
    bq_d = nc.declare_dram_parameter("bq", [P, MT], fp32, isOutput=False)
    bk_d = nc.declare_dram_parameter("bk", [P, MT], fp32, isOutput=False)
    bv_d = nc.declare_dram_parameter("bv", [1, GCOLS], bf16, isOutput=False)
    wgq_d = nc.declare_dram_parameter("wgq", [P, KC, HPC], fp32, isOutput=False)
    wgk_d = nc.declare_dram_parameter("wgk", [P, KC, HPC], fp32, isOutput=False)
    bg_d = nc.declare_dram_parameter("bg", [1, HPC], fp32, isOutput=False)
    mtri_d = nc.declare_dram_parameter("mtri", [P, P], bf16, isOutput=False)
    outp = nc.declare_dram_parameter("out", [S, DOUT], fp32, isOutput=True)

    scale = 1.0 / float(np.sqrt(DK))
    DK1 = DK + 1

    with tile.TileContext(nc) as tc:
        with (
            tc.tile_pool(name="persist", bufs=1) as pp,
            tc.tile_pool(name="wts", bufs=1) as wp,
        ):
            xq = pp.tile([P, KC, S], bf16, tag="xq")
            xk = pp.tile([P, KC, S], bf16, tag="xk")
            xv = pp.tile([P, KC, S], bf16, tag="xv")
            qt = pp.tile([P, MT, S], bf16, tag="qt")
            kt = pp.tile([P, MT, S], bf16, tag="kt")
            vaug = pp.tile([P, ST, HPC, DK1], bf16, tag="vaug")
            hcat = pp.tile([P, KC2, S], bf16, tag="hcat")
            ones = pp.tile([P, P], fp32, tag="ones")
            nc.any.memset(ones[:], 1.0)
            ones_bf = pp.tile([1, P], bf16, tag="ones_bf")
            nc.any.memset(ones_bf[:], 1.0)

            wq = wp.tile([P, KC, GCOLS], bf16, tag="wq")
            wk = wp.tile([P, KC, GCOLS], bf16, tag="wk")
            wv = wp.tile([P, KC, GCOLS], bf16, tag="wv")
            wo = wp.tile([P, KC2, DOUT], bf16, tag="wo")
            bq = wp.tile([P, MT], fp32, tag="bq")
            bk = wp.tile([P, MT], fp32, tag="bk")
            bv = wp.tile([1, GCOLS], bf16, tag="bv")
            wgq = wp.tile([P, KC, HPC], fp32, tag="wgq")
            wgk = wp.tile([P, KC, HPC], fp32, tag="wgk")
            bg = wp.tile([1, HPC], fp32, tag="bg")
            mtri = wp.tile([P, P], bf16, tag="mtri")

            # ---- DMA issue order: what the first matmuls need comes first,
            # each big tensor split across several queues.
            for c in range(0, KC, 2):
                nc.sync.dma_start(wq[:, c : c + 2, :], wq_d[:, c : c + 2, :])
            nc.sync.dma_start(bq[:], bq_d[:])
            nc.sync.dma_start(mtri[:], mtri_d[:])
            for c in range(0, KC, 2):
                nc.sync.dma_start(wk[:, c : c + 2, :], wk_d[:, c : c + 2, :])
            nc.sync.dma_start(bk[:], bk_d[:])
            for c in range(KC):
                nc.sync.dma_start(xq[:, c, 0:HALF], xq_d[:, c, 0:HALF])
            for c in range(KC):
                nc.sync.dma_start(xk[:, c, 0:HALF], xk_d[:, c, 0:HALF])
            for c in range(0, KC, 2):
                nc.sync.dma_start(wv[:, c : c + 2, :], wv_d[:, c : c + 2, :])
            nc.sync.dma_start(bv[:], bv_d[:])
            for c in range(KC):
                nc.sync.dma_start(xv[:, c, 0:HALF], xv_d[:, c, 0:HALF])
            for c in range(KC):
                nc.sync.dma_start(xq[:, c, HALF:S], xq_d[:, c, HALF:S])
            for c in range(KC):
                nc.sync.dma_start(xk[:, c, HALF:S], xk_d[:, c, HALF:S])
            for c in range(KC):
                nc.sync.dma_start(xv[:, c, HALF:S], xv_d[:, c, HALF:S])
            for c in range(KC2):
                nc.sync.dma_start(wo[:, c, :], wo_d[:, c, :])
            nc.sync.dma_start(wgq[:], wgq_d[:])
            nc.sync.dma_start(wgk[:], wgk_d[:])
            nc.sync.dma_start(bg[:], bg_d[:])

            # ---------------- Stage A: projections + pooled means + gate,
            # interleaved across Q/K/V by q-chunk to match DMA arrival.
            with (
                tc.tile_pool(name="psa", bufs=3, space="PSUM") as psa,
                tc.tile_pool(name="pmisc", bufs=1) as pm,
            ):
                pooled_q = pm.tile([P, KC], fp32, tag="pq")
                pooled_k = pm.tile([P, KC], fp32, tag="pk")
                scratch = pm.tile([P, S], bf16, tag="scratch")

                def proj_chunk(x_sb, w_sb, b_sb, out_sb, n):
                    nsl = ds(n * CHUNK, CHUNK)
                    for m in range(MT):
                        ps = psa.tile([P, CHUNK], fp32, tag="psa", bufs=3)
                        for k in range(KC):
                            nc.tensor.matmul(
                                ps[:], w_sb[:, k, ts(m, P)], x_sb[:, k, nsl],
                                start=(k == 0), stop=(k == KC - 1))
                        nc.vector.tensor_scalar_add(
                            out_sb[:, m, nsl], ps[:], b_sb[:, m : m + 1])

                def v_tile(st):
                    ps = psa.tile([P, GCOLS], fp32, tag="psv", bufs=2)
                    for k in range(KC):
                        nc.tensor.matmul(
                            ps[:], xv[:, k, ts(st, P)], wv[:, k, :],
                            start=(k == 0), stop=False)
                    nc.tensor.matmul(
                        ps[:], ones_bf[0:1, 0:P], bv[:], start=False, stop=True)
                    nc.vector.tensor_copy(
                        vaug[:, st, :, 0:DK],
                        ps.rearrange("p (h d) -> p h d", d=DK))

                for n in range(NCH):
                    proj_chunk(xq, wq, bq, qt, n)
                    proj_chunk(xk, wk, bk, kt, n)
                    if n == NCH - 1:
                        for k in range(KC):
                            nc.scalar.activation(
                                scratch[:], xq[:, k, :], Act.Identity,
                                accum_out=pooled_q[:, k : k + 1])
                        for k in range(KC):
                            nc.scalar.activation(
                                scratch[:], xk[:, k, :], Act.Identity,
                                accum_out=pooled_k[:, k : k + 1])
                    for st in range(n * TPC, (n + 1) * TPC):
                        v_tile(st)

                # gate logits -> sigmoid -> 1/g -> vaug column DK
                psg = psa.tile([1, HPC], fp32, tag="psg", bufs=1)
                for k in range(KC):
                    nc.tensor.matmul(psg[:], pooled_q[:, k : k + 1], wgq[:, k, :],
                                     start=(k == 0), stop=False)
                for k in range(KC):
                    nc.tensor.matmul(psg[:], pooled_k[:, k : k + 1], wgk[:, k, :],
                                     start=False, stop=False)
                nc.tensor.matmul(psg[:], ones[0:1, 0:1], bg[:],
                                 start=False, stop=True)
                gate0 = pm.tile([1, HPC], fp32, tag="gate0")
                nc.scalar.activation(gate0[:], psg[:], Act.Sigmoid)
                invg = pm.tile([1, ST, HPC], bf16, tag="invg")
                invgf = pm.tile([1, 1, HPC], fp32, tag="invgf")
                nc.vector.reciprocal(invgf[0:1, 0, :], gate0[0:1, :])
                nc.vector.tensor_copy(
                    invg[0:1, :, :], invgf[:].to_broadcast([1, ST, HPC]))
                nc.gpsimd.partition_broadcast(
                    vaug[:, :, :, DK:DK1], invg[0:1, :, :])

            # ---------------- Stage B + interleaved stage C
            with (
                tc.tile_pool(name="attn", bufs=3) as ap_,
                tc.tile_pool(name="rows", bufs=2) as rp,
                tc.tile_pool(name="otmp", bufs=2) as op_,
                tc.tile_pool(name="pssc", bufs=2, space="PSUM") as pssc,
                tc.tile_pool(name="psav", bufs=2, space="PSUM") as psav,
            ):
                blocks = []
                for j in range(NCH):
                    nkv_j = min(TPC * (j + 1), NKV) if causal else NKV
                    for hp in range(MT):
                        blocks.append((j, hp, nkv_j, ds(j * CHUNK, CHUNK)))

                def emit_av(item):
                    i, at, qrel, Ni, pe, po, hp, nkv_j = item
                    nc.tensor.matmul(
                        pe[:, ds(qrel, Ni)], vaug[:, i, 2 * hp, :],
                        at[:, 0, :Ni], start=(i == 0), stop=(i == nkv_j - 1))
                    nc.tensor.matmul(
                        po[:, ds(qrel, Ni)], vaug[:, i, 2 * hp + 1, :],
                        at[:, 1, :Ni], start=(i == 0), stop=(i == nkv_j - 1))

                def emit_chain(st8):
                    # den rows -> partitions 0/1 -> one [2,512] reciprocal ->
                    # ring broadcast; everything data-ready when reached.
                    (pe, po), _, _, _ = st8["av"]
                    rstk = rp.tile([P, 6, CHUNK], fp32, tag="rr", bufs=2)
                    for half, pav in ((0, pe), (1, po)):
                        nc.vector.tensor_copy(rstk[DK : DK1, half, :],
                                              pav[DK : DK1, :])
                        nc.sync.dma_start(rstk[half : half + 1, 2, :],
                                          rstk[DK : DK1, half, :])
                    nc.vector.reciprocal(rstk[0:2, 3, :], rstk[0:2, 2, :])
                    nc.sync.dma_start(rstk[0:1, 4, :], rstk[1:2, 3, :])
                    bc2 = rp.tile([DK, 2, CHUNK], fp32, tag="bcs", bufs=2)
                    nc.gpsimd.partition_broadcast(bc2[:], rstk[0:1, 3:5, :])
                    st8["bcs"] = bc2

                def emit_muls(st8):
                    (pe, po), hp, jsl, _ = st8["av"]
                    bc2 = st8["bcs"]
                    nc.vector.tensor_mul(hcat[0:DK, hp, jsl],
                                         pe[0:DK, :], bc2[:, 0, :])
                    ot = op_.tile([DK, CHUNK], bf16, tag="ot", bufs=2)
                    nc.vector.tensor_mul(ot[:], po[0:DK, :], bc2[:, 1, :])
                    nc.sync.dma_start(hcat[DK:P, hp, jsl], ot[:])

                hist = []      # per-block state dicts, newest last
                for bidx, (j, hp, nkv_j, jsl) in enumerate(blocks):
                    pe = psav.tile([DK1, CHUNK], fp32, tag="av_e", bufs=2)
                    po = psav.tile([DK1, CHUNK], fp32, tag="av_o", bufs=2)
                    st8 = {"av": ((pe, po), hp, jsl, nkv_j)}
                    avq = []
                    for i in range(nkv_j):
                        t = i - TPC * j
                        diag = causal and t >= 0
                        if diag:
                            Ni = CHUNK - P * t
                            qoff = j * CHUNK + P * t
                        else:
                            Ni = CHUNK
                            qoff = j * CHUNK
                        sc = pssc.tile([P, 2, CHUNK], fp32, tag="sc", bufs=2)
                        nc.tensor.matmul(
                            sc[:, 0, :Ni], kt[0:DK, hp, ts(i, P)],
                            qt[0:DK, hp, ds(qoff, Ni)], start=True, stop=True)
                        nc.tensor.matmul(
                            sc[:, 1, :Ni], kt[DK:P, hp, ts(i, P)],
                            qt[DK:P, hp, ds(qoff, Ni)], start=True, stop=True)
                        at = ap_.tile([P, 2, CHUNK], bf16, tag="at", bufs=3)
                        nc.scalar.activation(at[:, :, :Ni], sc[:, :, :Ni],
                                             Act.Exp, scale=scale)
                        if diag:
                            nc.vector.tensor_mul(
                                at[:, 0, 0:P], at[:, 0, 0:P], mtri[:])
                            nc.vector.tensor_mul(
                                at[:, 1, 0:P], at[:, 1, 0:P], mtri[:])
                        if len(avq) == 1:
                            emit_av(avq.pop(0))
                        avq.append((i, at, qoff - j * CHUNK, Ni,
                                    pe, po, hp, nkv_j))
                    while avq:
                        emit_av(avq.pop(0))
                    emit_chain(st8)
                    if hist:
                        emit_muls(hist[-1])
                    hist.append(st8)
                emit_muls(hist[-1])

            # ---------------- Stage C: output projection (host adds bo)
            with (
                tc.tile_pool(name="osb2", bufs=3) as ob,
                tc.tile_pool(name="psoc", bufs=3, space="PSUM") as psoc,
            ):
                for st in range(ST):
                    osb = ob.tile([P, DOUT], fp32, tag="osb", bufs=3)
                    for nh in range(NOC):
                        ps = psoc.tile([P, CHUNK], fp32, tag="pso", bufs=3)
                        for k2 in range(KC2):
                            nc.tensor.matmul(
                                ps[:], hcat[:, k2, ts(st, P)],
                                wo[:, k2, ds(nh * CHUNK, CHUNK)],
                                start=(k2 == 0), stop=(k2 == KC2 - 1))
                        if nh == 0:
                            nc.scalar.copy(osb[:, 0:CHUNK], ps[:])
                        else:
                            nc.vector.tensor_copy(osb[:, CHUNK:DOUT], ps[:])
                    nc.sync.dma_start(outp[ts(st, P), :], osb[:])

    nc.compile()
    return nc


def _prep_core_inputs(query, key_, value, Wq, bq, Wk, bk, Wv, bv, Wg, bg, Wo,
                      b, g, S, D, HPC, DK):
    import ml_dtypes
    GCOLS = HPC * DK
    KC = D // P
    KC2 = GCOLS // P
    MT = GCOLS // P
    H0 = g * HPC
    cs = slice(H0 * DK, H0 * DK + GCOLS)
    f32 = np.float32
    bf16 = ml_dtypes.bfloat16
    fp8 = ml_dtypes.float8_e4m3
    c = np.ascontiguousarray

    def shuf_x(x, dt):
        # [S, D] -> [P, KC, S] with [p, k, s] = x[s, k*P+p]
        return c(x.T.reshape(KC, P, S).transpose(1, 0, 2).astype(dt))

    def shuf_w(W):
        # [D, GCOLS] -> [P, KC, GCOLS]
        return c(W.reshape(KC, P, -1).transpose(1, 0, 2).astype(bf16))

    return {
        "xq": shuf_x(query[b], bf16),
        "xk": shuf_x(key_[b], bf16),
        "xv": shuf_x(value[b], bf16),
        "wq": shuf_w(Wq[:, cs]),
        "wk": shuf_w(Wk[:, cs]),
        "wv": shuf_w(Wv[:, cs]),
        "wo": c(Wo[cs, :].reshape(KC2, P, -1).transpose(1, 0, 2).astype(bf16)),
        "bq": c(bq[cs].reshape(MT, P).T.astype(f32)),
        "bk": c(bk[cs].reshape(MT, P).T.astype(f32)),
        "bv": c(bv[cs].astype(bf16)[None, :]),
        "wgq": c((Wg[:D, H0 : H0 + HPC] / S).reshape(KC, P, HPC)
                 .transpose(1, 0, 2).astype(f32)),
        "wgk": c((Wg[D:, H0 : H0 + HPC] / S).reshape(KC, P, HPC)
                 .transpose(1, 0, 2).astype(f32)),
        "bg": c(bg[H0 : H0 + HPC].astype(f32)[None, :]),
        "mtri": np.triu(np.ones((P, P), bf16)),
    }


_last_results = None


def kernel(query, key_, value, mask, Wq, bq, Wk, bk, Wv, bv, Wo, bo, Wg, bg):
    global _last_results
    from concourse.bass_utils import run_bass_kernel_spmd

    query = np.asarray(query)
    key_ = np.asarray(key_)
    value = np.asarray(value)
    mask = np.asarray(mask)
    B, S, D = query.shape
    H = np.asarray(bg).shape[0]
    DK = D // H
    DOUT = np.asarray(Wo).shape[1]
    NC_ = 8
    GROUPS = NC_ // B
    HPC = H // GROUPS

    causal = bool(
        np.array_equal(mask[0, 0], np.tril(np.ones((S, S), bool)))
    )
    if not causal:
        assert mask.all(), "only causal or all-true masks supported"

    key = (S, D, DOUT, HPC, DK, causal)
    if key not in _BUILD_CACHE:
        _BUILD_CACHE[key] = _build(*key)
    nc = _BUILD_CACHE[key]

    in_maps = []
    for cidx in range(NC_):
        b, gidx = divmod(cidx, GROUPS)
        in_maps.append(_prep_core_inputs(
            query, key_, value, Wq, bq, Wk, bk, Wv, bv, Wg, bg, Wo,
            b, gidx, S, D, HPC, DK))

    res = run_bass_kernel_spmd(nc, in_maps, core_ids=list(range(NC_)))
    _last_results = res

    out = np.zeros((B, S, DOUT), np.float32)
    for cidx in range(NC_):
        b = cidx // GROUPS
        out[b] += res.results[cidx]["out"]
    out += np.asarray(bo).astype(np.float32)
    return out


# revision 45
# speedup vs baseline: 1.1793x; 1.0383x over previous
"""Trainium2 Bass kernel for nn_MultiHeadAttention (B=2, S=2048, D=1024, H=16).

Sharding: 8 cores = 2 batches x 4 head-groups (4 heads each).
Each core receives host-preshuffled activations x^T ([P, KC, S] layout,
fp8e4 to halve the input stream) plus its head-group's weight slices (bf16).

Per core:
  stage A (interleaved by q-chunk with the input DMA stream):
    Q^T,K^T = W^T x^T  (per-head [DK, S] bf16, head pair stacked on parts)
    V_aug   = [x Wv + bv, 1/g]  (natural [S, DK+1] per head; column DK holds
      the reciprocal of the per-head sigmoid gate, device-computed and
      written once via gpsimd partition_broadcast)
    pooled means on DVE; gate = sigmoid(pooled @ Wg)
  stage B: flat stream of (q-chunk, head-pair) blocks with a global
    2-deep AV software pipeline that crosses block boundaries:
      scores: two contract-64 matmuls as concurrent PE row-tiles into two
        banks of one PSUM tile -> ONE joint Exp [128, 2, Ni] on scalar;
        causal diagonal masked on DVE.
      normalization chain for block k emitted at block k+1 step 1
        (copies->hop->one [2,512] reciprocal->gpsimd ring broadcast),
        multiplies at block k+2 step 1, stage C s-tile k at block k+3 end.
    gpsimd runs ONLY partition_broadcast (no ucode library thrash).
  stage C: interleaved output projection (host sums 4 partials + bo).
"""

import numpy as np

P = 128
CHUNK = 512

_BUILD_CACHE = {}


def _build(S, D, DOUT, HPC, DK, causal):
    import concourse.bass as bass
    import concourse.mybir as mybir
    import concourse.tile as tile
    from concourse import bacc
    from concourse.bass import ds, ts

    fp32 = mybir.dt.float32
    bf16 = mybir.dt.bfloat16
    fp8 = mybir.dt.float8e4
    KC = D // P             # contraction k-chunks for projections
    GCOLS = HPC * DK        # this core's projection output width
    MT = GCOLS // P         # head-pair tiles (2 heads of DK=64 per tile)
    NCH = S // CHUNK        # q-chunks
    TPC = CHUNK // P        # kv tiles per q-chunk (4)
    NKV = S // P            # kv tiles total
    KC2 = GCOLS // P        # out-proj contraction chunks
    NOC = DOUT // CHUNK     # out-proj N chunks
    ST = S // P             # s-tiles
    HALF = S // 2
    assert DK * 2 == P and GCOLS % P == 0 and NCH % 2 == 0
    assert NOC == 2, "stage C packs its two N-chunks into one 2-bank tile"

    Act = mybir.ActivationFunctionType
    nc = bacc.Bacc()

    xq_d = nc.declare_dram_parameter("xq", [P, KC, S], bf16, isOutput=False)
    xk_d = nc.declare_dram_parameter("xk", [P, KC, S], bf16, isOutput=False)
    xv_d = nc.declare_dram_parameter("xv", [P, KC, S], bf16, isOutput=False)
    wq_d = nc.declare_dram_parameter("wq", [P, KC, GCOLS], bf16, isOutput=False)
    wk_d = nc.declare_dram_parameter("wk", [P, KC, GCOLS], bf16, isOutput=False)
    wv_d = nc.declare_dram_parameter("wv", [P, KC, GCOLS], bf16, isOutput=False)
    wo_d = nc.declare_dram_parameter("wo", [P, KC2, DOUT], bf16, isOutput=False)
    bq_d = nc.declare_dram_parameter("bq", [P, MT], fp32, isOutput=False)
    bk_d = nc.declare_dram_parameter("bk", [P, MT], fp32, isOutput=False)
    bv_d = nc.declare_dram_parameter("bv", [1, GCOLS], bf16, isOutput=False)
    wgq_d = nc.declare_dram_parameter("wgq", [P, KC, HPC], fp32, isOutput=False)
    wgk_d = nc.declare_dram_parameter("wgk", [P, KC, HPC], fp32, isOutput=False)
    bg_d = nc.declare_dram_parameter("bg", [1, HPC], fp32, isOutput=False)
    mtri_d = nc.declare_dram_parameter("mtri", [P, P], bf16, isOutput=False)
    outp = nc.declare_dram_parameter("out", [S, DOUT], bf16, isOutput=True)

    scale = 1.0 / float(np.sqrt(DK))
    DK1 = DK + 1

    with tile.TileContext(nc) as tc:
        with (
            tc.tile_pool(name="persist", bufs=1) as pp,
            tc.tile_pool(name="wts", bufs=1) as wp,
        ):
            xq = pp.tile([P, KC, S], bf16, tag="xq")
            xk = pp.tile([P, KC, S], bf16, tag="xk")
            xv = pp.tile([P, KC, S], bf16, tag="xv")
            qt = pp.tile([P, MT, S], bf16, tag="qt")
            kt = pp.tile([P, MT, S], bf16, tag="kt")
            vaug = pp.tile([P, ST, HPC, DK1], bf16, tag="vaug")
            hcat = pp.tile([P, KC2, S], bf16, tag="hcat")
            ones = pp.tile([P, P], fp32, tag="ones")
            nc.any.memset(ones[:], 1.0)
            ones_bf = pp.tile([1, P], bf16, tag="ones_bf")
            nc.any.memset(ones_bf[:], 1.0)

            wq = wp.tile([P, KC, GCOLS], bf16, tag="wq")
            wk = wp.tile([P, KC, GCOLS], bf16, tag="wk")
            wv = wp.tile([P, KC, GCOLS], bf16, tag="wv")
            wo = wp.tile([P, KC2, DOUT], bf16, tag="wo")
            bq = wp.tile([P, MT], fp32, tag="bq")
            bk = wp.tile([P, MT], fp32, tag="bk")
            bv = wp.tile([1, GCOLS], bf16, tag="bv")
            wgq = wp.tile([P, KC, HPC], fp32, tag="wgq")
            wgk = wp.tile([P, KC, HPC], fp32, tag="wgk")
            bg = wp.tile([1, HPC], fp32, tag="bg")
            mtri = wp.tile([P, P], bf16, tag="mtri")

            # ---- DMA issue order: what the first matmuls need comes first,
            # each big tensor split across several queues.
            for c in range(0, KC, 2):
                nc.sync.dma_start(wq[:, c : c + 2, :], wq_d[:, c : c + 2, :])
            nc.sync.dma_start(bq[:], bq_d[:])
            nc.sync.dma_start(mtri[:], mtri_d[:])
            for c in range(0, KC, 2):
                nc.sync.dma_start(wk[:, c : c + 2, :], wk_d[:, c : c + 2, :])
            nc.sync.dma_start(bk[:], bk_d[:])
            for c in range(KC):
                nc.sync.dma_start(xq[:, c, 0:CHUNK], xq_d[:, c, 0:CHUNK])
            for c in range(KC):
                nc.sync.dma_start(xq[:, c, CHUNK:HALF], xq_d[:, c, CHUNK:HALF])
            for c in range(KC):
                nc.sync.dma_start(xk[:, c, 0:HALF], xk_d[:, c, 0:HALF])
            for c in range(0, KC, 2):
                nc.sync.dma_start(wv[:, c : c + 2, :], wv_d[:, c : c + 2, :])
            nc.sync.dma_start(bv[:], bv_d[:])
            for c in range(KC):
                nc.sync.dma_start(xv[:, c, 0:HALF], xv_d[:, c, 0:HALF])
            for c in range(KC):
                nc.sync.dma_start(xq[:, c, HALF:S], xq_d[:, c, HALF:S])
            for c in range(KC):
                nc.sync.dma_start(xk[:, c, HALF:S], xk_d[:, c, HALF:S])
            for c in range(KC):
                nc.sync.dma_start(xv[:, c, HALF:S], xv_d[:, c, HALF:S])
            for c in range(KC2):
                nc.sync.dma_start(wo[:, c, :], wo_d[:, c, :])
            nc.sync.dma_start(wgq[:], wgq_d[:])
            nc.sync.dma_start(wgk[:], wgk_d[:])
            nc.sync.dma_start(bg[:], bg_d[:])

            # ---------------- Stage A: projections + pooled means + gate,
            # interleaved across Q/K/V by q-chunk to match DMA arrival.
            with (
                tc.tile_pool(name="psa", bufs=3, space="PSUM") as psa,
                tc.tile_pool(name="pmisc", bufs=1) as pm,
            ):
                pooled_q = pm.tile([P, KC], fp32, tag="pq")
                pooled_k = pm.tile([P, KC], fp32, tag="pk")
                scratch = pm.tile([P, S], bf16, tag="scratch")

                def proj_chunk(x_sb, w_sb, b_sb, out_sb, n):
                    nsl = ds(n * CHUNK, CHUNK)
                    for m in range(MT):
                        ps = psa.tile([P, CHUNK], fp32, tag="psa", bufs=3)
                        for k in range(KC):
                            nc.tensor.matmul(
                                ps[:], w_sb[:, k, ts(m, P)], x_sb[:, k, nsl],
                                start=(k == 0), stop=(k == KC - 1))
                        nc.vector.tensor_scalar_add(
                            out_sb[:, m, nsl], ps[:], b_sb[:, m : m + 1])

                def v_tile(st):
                    ps = psa.tile([P, GCOLS], fp32, tag="psv", bufs=2)
                    for k in range(KC):
                        nc.tensor.matmul(
                            ps[:], xv[:, k, ts(st, P)], wv[:, k, :],
                            start=(k == 0), stop=False)
                    nc.tensor.matmul(
                        ps[:], ones_bf[0:1, 0:P], bv[:], start=False, stop=True)
                    nc.vector.tensor_copy(
                        vaug[:, st, :, 0:DK],
                        ps.rearrange("p (h d) -> p h d", d=DK))

                for n in range(NCH):
                    proj_chunk(xq, wq, bq, qt, n)
                    proj_chunk(xk, wk, bk, kt, n)
                    if n == NCH - 1:
                        for k in range(KC):
                            nc.scalar.activation(
                                scratch[:], xq[:, k, :], Act.Identity,
                                accum_out=pooled_q[:, k : k + 1])
                        for k in range(KC):
                            nc.scalar.activation(
                                scratch[:], xk[:, k, :], Act.Identity,
                                accum_out=pooled_k[:, k : k + 1])
                    for st in range(n * TPC, (n + 1) * TPC):
                        v_tile(st)

                # gate logits -> sigmoid -> 1/g -> vaug column DK
                psg = psa.tile([1, HPC], fp32, tag="psg", bufs=1)
                for k in range(KC):
                    nc.tensor.matmul(psg[:], pooled_q[:, k : k + 1], wgq[:, k, :],
                                     start=(k == 0), stop=False)
                for k in range(KC):
                    nc.tensor.matmul(psg[:], pooled_k[:, k : k + 1], wgk[:, k, :],
                                     start=False, stop=False)
                nc.tensor.matmul(psg[:], ones[0:1, 0:1], bg[:],
                                 start=False, stop=True)
                gate0 = pm.tile([1, HPC], fp32, tag="gate0")
                nc.scalar.activation(gate0[:], psg[:], Act.Sigmoid)
                invg = pm.tile([1, ST, HPC], bf16, tag="invg")
                invgf = pm.tile([1, 1, HPC], fp32, tag="invgf")
                nc.vector.reciprocal(invgf[0:1, 0, :], gate0[0:1, :])
                nc.vector.tensor_copy(
                    invg[0:1, :, :], invgf[:].to_broadcast([1, ST, HPC]))
                nc.gpsimd.partition_broadcast(
                    vaug[:, :, :, DK:DK1], invg[0:1, :, :])

            # ---------------- Stage B + interleaved stage C
            with (
                tc.tile_pool(name="attn", bufs=3) as ap_,
                tc.tile_pool(name="rows", bufs=2) as rp,
                tc.tile_pool(name="otmp", bufs=2) as op_,
                tc.tile_pool(name="pssc", bufs=2, space="PSUM") as pssc,
                tc.tile_pool(name="psav", bufs=2, space="PSUM") as psav,
            ):
                blocks = []
                for j in range(NCH):
                    nkv_j = min(TPC * (j + 1), NKV) if causal else NKV
                    for hp in range(MT):
                        blocks.append((j, hp, nkv_j, ds(j * CHUNK, CHUNK)))

                def emit_av(item):
                    i, at, qrel, Ni, pe, po, hp, nkv_j = item
                    nc.tensor.matmul(
                        pe[:, ds(qrel, Ni)], vaug[:, i, 2 * hp, :],
                        at[:, 0, :Ni], start=(i == 0), stop=(i == nkv_j - 1))
                    nc.tensor.matmul(
                        po[:, ds(qrel, Ni)], vaug[:, i, 2 * hp + 1, :],
                        at[:, 1, :Ni], start=(i == 0), stop=(i == nkv_j - 1))

                def emit_chain(st8):
                    # den rows -> partitions 0/1 -> one [2,512] reciprocal ->
                    # ring broadcast; everything data-ready when reached.
                    (pe, po), _, _, _ = st8["av"]
                    rstk = rp.tile([P, 6, CHUNK], fp32, tag="rr", bufs=2)
                    for half, pav in ((0, pe), (1, po)):
                        nc.vector.tensor_copy(rstk[DK : DK1, half, :],
                                              pav[DK : DK1, :])
                        nc.sync.dma_start(rstk[half : half + 1, 2, :],
                                          rstk[DK : DK1, half, :])
                    nc.vector.reciprocal(rstk[0:2, 3, :], rstk[0:2, 2, :])
                    nc.sync.dma_start(rstk[0:1, 4, :], rstk[1:2, 3, :])
                    bc2 = rp.tile([DK, 2, CHUNK], fp32, tag="bcs", bufs=2)
                    nc.gpsimd.partition_broadcast(bc2[:], rstk[0:1, 3:5, :])
                    st8["bcs"] = bc2

                def emit_muls(st8):
                    (pe, po), hp, jsl, _ = st8["av"]
                    bc2 = st8["bcs"]
                    nc.vector.tensor_mul(hcat[0:DK, hp, jsl],
                                         pe[0:DK, :], bc2[:, 0, :])
                    ot = op_.tile([DK, CHUNK], bf16, tag="ot", bufs=2)
                    nc.vector.tensor_mul(ot[:], po[0:DK, :], bc2[:, 1, :])
                    nc.sync.dma_start(hcat[DK:P, hp, jsl], ot[:])

                hist = []      # per-block state dicts, newest last
                for bidx, (j, hp, nkv_j, jsl) in enumerate(blocks):
                    pe = psav.tile([DK1, CHUNK], fp32, tag="av_e", bufs=2)
                    po = psav.tile([DK1, CHUNK], fp32, tag="av_o", bufs=2)
                    st8 = {"av": ((pe, po), hp, jsl, nkv_j)}
                    avq = []
                    for i in range(nkv_j):
                        t = i - TPC * j
                        diag = causal and t >= 0
                        if diag:
                            Ni = CHUNK - P * t
                            qoff = j * CHUNK + P * t
                        else:
                            Ni = CHUNK
                            qoff = j * CHUNK
                        sc = pssc.tile([P, 2, CHUNK], fp32, tag="sc", bufs=2)
                        nc.tensor.matmul(
                            sc[:, 0, :Ni], kt[0:DK, hp, ts(i, P)],
                            qt[0:DK, hp, ds(qoff, Ni)], start=True, stop=True)
                        nc.tensor.matmul(
                            sc[:, 1, :Ni], kt[DK:P, hp, ts(i, P)],
                            qt[DK:P, hp, ds(qoff, Ni)], start=True, stop=True)
                        at = ap_.tile([P, 2, CHUNK], bf16, tag="at", bufs=4)
                        nc.scalar.activation(at[:, :, :Ni], sc[:, :, :Ni],
                                             Act.Exp, scale=scale)
                        if diag:
                            nc.vector.tensor_mul(
                                at[:, 0, 0:P], at[:, 0, 0:P], mtri[:])
                            nc.vector.tensor_mul(
                                at[:, 1, 0:P], at[:, 1, 0:P], mtri[:])
                        if len(avq) == 1:
                            emit_av(avq.pop(0))
                        avq.append((i, at, qoff - j * CHUNK, Ni,
                                    pe, po, hp, nkv_j))
                    while avq:
                        emit_av(avq.pop(0))
                    emit_chain(st8)
                    if hist:
                        emit_muls(hist[-1])
                    hist.append(st8)
                emit_muls(hist[-1])

            # ---------------- Stage C: output projection (host adds bo)
            with (
                tc.tile_pool(name="osb2", bufs=3) as ob,
                tc.tile_pool(name="psoc", bufs=3, space="PSUM") as psoc,
            ):
                for st in range(ST):
                    osb = ob.tile([P, DOUT], bf16, tag="osb", bufs=3)
                    for nh in range(NOC):
                        ps = psoc.tile([P, CHUNK], fp32, tag="pso", bufs=3)
                        for k2 in range(KC2):
                            nc.tensor.matmul(
                                ps[:], hcat[:, k2, ts(st, P)],
                                wo[:, k2, ds(nh * CHUNK, CHUNK)],
                                start=(k2 == 0), stop=(k2 == KC2 - 1))
                        if nh == 0:
                            nc.scalar.copy(osb[:, 0:CHUNK], ps[:])
                        else:
                            nc.vector.tensor_copy(osb[:, CHUNK:DOUT], ps[:])
                    nc.sync.dma_start(outp[ts(st, P), :], osb[:])

    nc.compile()
    return nc


def _prep_core_inputs(query, key_, value, Wq, bq, Wk, bk, Wv, bv, Wg, bg, Wo,
                      b, g, S, D, HPC, DK):
    import ml_dtypes
    GCOLS = HPC * DK
    KC = D // P
    KC2 = GCOLS // P
    MT = GCOLS // P
    H0 = g * HPC
    cs = slice(H0 * DK, H0 * DK + GCOLS)
    f32 = np.float32
    bf16 = ml_dtypes.bfloat16
    fp8 = ml_dtypes.float8_e4m3
    c = np.ascontiguousarray

    def shuf_x(x, dt):
        # [S, D] -> [P, KC, S] with [p, k, s] = x[s, k*P+p]
        return c(x.T.reshape(KC, P, S).transpose(1, 0, 2).astype(dt))

    def shuf_w(W):
        # [D, GCOLS] -> [P, KC, GCOLS]
        return c(W.reshape(KC, P, -1).transpose(1, 0, 2).astype(bf16))

    return {
        "xq": shuf_x(query[b], bf16),
        "xk": shuf_x(key_[b], bf16),
        "xv": shuf_x(value[b], bf16),
        "wq": shuf_w(Wq[:, cs]),
        "wk": shuf_w(Wk[:, cs]),
        "wv": shuf_w(Wv[:, cs]),
        "wo": c(Wo[cs, :].reshape(KC2, P, -1).transpose(1, 0, 2).astype(bf16)),
        "bq": c(bq[cs].reshape(MT, P).T.astype(f32)),
        "bk": c(bk[cs].reshape(MT, P).T.astype(f32)),
        "bv": c(bv[cs].astype(bf16)[None, :]),
        "wgq": c((Wg[:D, H0 : H0 + HPC] / S).reshape(KC, P, HPC)
                 .transpose(1, 0, 2).astype(f32)),
        "wgk": c((Wg[D:, H0 : H0 + HPC] / S).reshape(KC, P, HPC)
                 .transpose(1, 0, 2).astype(f32)),
        "bg": c(bg[H0 : H0 + HPC].astype(f32)[None, :]),
        "mtri": np.triu(np.ones((P, P), bf16)),
    }


_last_results = None


def kernel(query, key_, value, mask, Wq, bq, Wk, bk, Wv, bv, Wo, bo, Wg, bg):
    global _last_results
    from concourse.bass_utils import run_bass_kernel_spmd

    query = np.asarray(query)
    key_ = np.asarray(key_)
    value = np.asarray(value)
    mask = np.asarray(mask)
    B, S, D = query.shape
    H = np.asarray(bg).shape[0]
    DK = D // H
    DOUT = np.asarray(Wo).shape[1]
    NC_ = 8
    GROUPS = NC_ // B
    HPC = H // GROUPS

    causal = bool(
        np.array_equal(mask[0, 0], np.tril(np.ones((S, S), bool)))
    )
    if not causal:
        assert mask.all(), "only causal or all-true masks supported"

    key = (S, D, DOUT, HPC, DK, causal)
    if key not in _BUILD_CACHE:
        _BUILD_CACHE[key] = _build(*key)
    nc = _BUILD_CACHE[key]

    in_maps = []
    for cidx in range(NC_):
        b, gidx = divmod(cidx, GROUPS)
        in_maps.append(_prep_core_inputs(
            query, key_, value, Wq, bq, Wk, bk, Wv, bv, Wg, bg, Wo,
            b, gidx, S, D, HPC, DK))

    res = run_bass_kernel_spmd(nc, in_maps, core_ids=list(range(NC_)))
    _last_results = res

    out = np.zeros((B, S, DOUT), np.float32)
    for cidx in range(NC_):
        b = cidx // GROUPS
        out[b] += res.results[cidx]["out"].astype(np.float32)
    out += np.asarray(bo).astype(np.float32)
    return out
